# revision 1
# baseline (speedup 1.0000x reference)
"""Trainium2 Bass kernel for the ODE-Multistep problem.

Math reformulation (exact): the reference re-evaluates the tiny MLP on a
3-step sliding window every scan iteration, so each time index t is pushed
through the MLP 3 times.  Writing g(t) = W3.T @ relu(W2.T @ relu(W1.T @
f(t))) with f(t) the 8 feature channels at time t, the scan is

    ni[j] = ni[j-1] + a2*g(j-1) + a1*g(j-2) + a0*g(j-3) + K0,   j = 3..511

with a_s = H*Ws[s], K0 = H*(b3*sum(Ws) + bs).  Each g(t) is computed once
(3x compute saving).  On device the combination a2*g(j-1)+a1*g(j-2)+
a0*g(j-3)+K0 is accumulated directly in PSUM by 3 phase-rotated
accumulator rows fed from the last matmul (W3 contraction), so the only
per-step non-matmul work on the sequential critical path is one
tensor_add (ni) and one Square activation (ni^2).

Layout: feature-on-partition ("transposed") activations, batch on the
free axis.  Per core: 1024 samples, chunks of 512 (PSUM bank width).
6 static feature channels are precomputed on device into DRAM
(Fdram[t] = [nr, nr(+1)-nr(-1), k, nr^2, k*nr, nr*(nr(+1)-nr(-1))]) and
streamed in during the scan; the 2 recurrent channels (ni, ni^2) live in
SBUF partitions 32/33 and enter the first matmul as a separate K=2
accumulating matmul (row-group 32, packable with the K=6 static matmul).

Sharding: data-parallel over batch, 8 cores x 1024 samples, weights
replicated, no cross-core traffic.
"""

import os
import sys
import numpy as np

sys.path.insert(0, "/opt/trn_rl_repo")

import concourse.bass as bass
import concourse.bacc as bacc
import concourse.mybir as mybir
from concourse import tile

FP = mybir.dt.float32
H = 1e-3
B_TOT = 8192
NCORES = 8
BC = B_TOT // NCORES          # 1024 samples per core
T_NR = 513
NT_OUT = 512                  # output time steps (cols of output)
CH = 512                      # psum chunk (free) size
NCHUNK = BC // CH

# float32r: hardware single-pass fp32 matmul mode, 4x faster than fp32
USE_F32R = True
DBG_SKIP = set(os.environ.get("KERNEL_DBG_SKIP", "").split(","))


def build_program(nt_out=NT_OUT, bc=BC, f32r=None):
    """Build the single-core SPMD Bass program.

    nt_out/bc reducible for simulator smoke tests.
    """
    if f32r is None:
        f32r = USE_F32R
    MMD = mybir.dt.float32r if f32r else FP
    # delta stage runs bf16: f32r matmuls cannot write PSUM at base
    # partition 32/64 (walrus ISA check), bf16 can; cost is N-cycles either
    # way and the W3 contraction is precision-insensitive.
    DLT = mybir.dt.bfloat16 if f32r else FP

    nsteps = nt_out - 1           # g(t) computed for t = 0..nt_out-2
    nchunk = bc // CH if bc >= CH else 1
    ch = CH if bc >= CH else bc

    nc = bacc.Bacc()

    # ---- I/O ----
    nrT = nc.declare_dram_parameter("nrT", [T_NR, bc], FP, isOutput=False)
    ivT = nc.declare_dram_parameter("ivT", [3, bc], MMD, isOutput=False)
    ivF = nc.declare_dram_parameter("ivF", [3, bc], FP, isOutput=False)
    kbd = nc.declare_dram_parameter("kb", [128, bc], FP, isOutput=False)
    w1s_d = nc.declare_dram_parameter("W1stat", [6, 200], MMD, isOutput=False)
    w1dn_d = nc.declare_dram_parameter("W1dni", [1, 200], MMD, isOutput=False)
    w1dq_d = nc.declare_dram_parameter("W1dni2", [1, 200], MMD, isOutput=False)
    w2k0_d = nc.declare_dram_parameter("W2k0", [128, 200], MMD, isOutput=False)
    w2k1_d = nc.declare_dram_parameter("W2k1", [72, 200], MMD, isOutput=False)
    b1a_d = nc.declare_dram_parameter("b1a", [128, 1], FP, isOutput=False)
    b1b_d = nc.declare_dram_parameter("b1b", [72, 1], FP, isOutput=False)
    b2a_d = nc.declare_dram_parameter("b2a", [128, 1], FP, isOutput=False)
    b2b_d = nc.declare_dram_parameter("b2b", [72, 1], FP, isOutput=False)
    ak0_d = nc.declare_dram_parameter("Ak0", [128, 3], DLT, isOutput=False)
    ak1_d = nc.declare_dram_parameter("Ak1", [72, 3], DLT, isOutput=False)
    k0_d = nc.declare_dram_parameter("K0v", [1, 1], FP, isOutput=False)
    sm_d = [[nc.declare_dram_parameter(f"SM{r}k{k}", [(128, 72)[k], (33, 65, 33)[r]],
                                       DLT, isOutput=False)
             for k in range(2)] for r in range(3)]
    outT = nc.declare_dram_parameter("outT", [nt_out, bc], MMD, isOutput=True)

    fdram = nc.dram_tensor("Fdram", [nsteps, 6, bc], MMD)

    with tile.TileContext(nc) as tc:
        with (
            tc.tile_pool(name="const", bufs=1) as constp,
            tc.tile_pool(name="state", bufs=1) as statep,
            tc.tile_pool(name="fpool", bufs=6) as fpool,
            tc.tile_pool(name="pre", bufs=3) as prep,
            tc.tile_pool(name="h1psum", bufs=3, space="PSUM") as h1pp,
            tc.tile_pool(name="h2psum", bufs=3, space="PSUM") as h2pp,
            tc.tile_pool(name="dpsum", bufs=1, space="PSUM") as dpp,
        ):
            # ---- persistent SBUF ----
            w1s = constp.tile([6, 200], MMD)
            w1dn = constp.tile([1, 200], MMD)
            w1dq = constp.tile([1, 200], MMD)
            w2k0 = constp.tile([128, 200], MMD)
            w2k1 = constp.tile([72, 200], MMD)
            b1a = constp.tile([128, 1], FP)
            b1b = constp.tile([72, 1], FP)
            b2a = constp.tile([128, 1], FP)
            b2b = constp.tile([72, 1], FP)
            ak0 = constp.tile([128, 3], DLT)
            ak1 = constp.tile([72, 3], DLT)
            k0t = constp.tile([1, 1], FP)
            sm = [[constp.tile([(128, 72)[k], (33, 65, 33)[r]], DLT,
                               name=f"sm{r}k{k}", tag=f"sm{r}k{k}")
                   for k in range(2)] for r in range(3)]
            zb32 = constp.tile([1, 1], FP)
            kb = constp.tile([128, bc], FP)
            zer = constp.tile([128, ch], FP)

            nc.sync.dma_start(w1s[:], w1s_d[:])
            nc.sync.dma_start(w1dn[0:1, :], w1dn_d[:])
            nc.sync.dma_start(w1dq[0:1, :], w1dq_d[:])
            nc.sync.dma_start(w2k0[:], w2k0_d[:])
            nc.sync.dma_start(w2k1[:], w2k1_d[:])
            nc.sync.dma_start(b1a[:], b1a_d[:])
            nc.sync.dma_start(b1b[:], b1b_d[:])
            nc.sync.dma_start(b2a[:], b2a_d[:])
            nc.sync.dma_start(b2b[:], b2b_d[:])
            nc.sync.dma_start(ak0[:], ak0_d[:])
            nc.sync.dma_start(ak1[:], ak1_d[:])
            nc.sync.dma_start(k0t[0:1, :], k0_d[:])
            for r in range(3):
                for k in range(2):
                    nc.sync.dma_start(sm[r][k][:], sm_d[r][k][:])
            nc.vector.memset(zb32[:], 0.0)
            nc.sync.dma_start(kb[:], kbd[:])
            nc.vector.memset(zer[:], 0.0)

            # ---- ni state: ni at partition 32, ni^2 at partition 64 ----
            nit = [statep.tile([1, bc], MMD, name=f"nit{i}", tag=f"nit{i}")
                   for i in range(2)]
            niq = [statep.tile([1, bc], MMD, name=f"niq{i}", tag=f"niq{i}")
                   for i in range(2)]

            # ---- h1T / h2T activations (persistent, rewritten per step) ----
            h1tp = [[statep.tile([128, bc], MMD, name=f"h1t0_{i}",
                                 tag=f"h1t0_{i}"),
                     statep.tile([72, bc], MMD, name=f"h1t1_{i}",
                                 tag=f"h1t1_{i}")] for i in range(2)]
            h2tp = [[statep.tile([128, bc], DLT, name=f"h2t0_{i}",
                                 tag=f"h2t0_{i}"),
                     statep.tile([72, bc], DLT, name=f"h2t1_{i}",
                                 tag=f"h2t1_{i}")] for i in range(2)]

            # delta accumulator: 3 phase rows
            # phase p lives at partition 32*p (matmul out base must be 0/32/64)
            dlt = dpp.tile([65, nchunk * ch], FP)

            # ---- phase 1: precompute static features into Fdram ----
            # engine APs need 32-aligned base partitions, so the +-1 shifted
            # views are materialized by three separate (overlapping) DMAs.
            for blk in range((nsteps + 127) // 128):
                t0 = blk * 128
                tn = min(128, nsteps - t0)
                am = prep.tile([128, bc], FP, tag="AM")   # nrT[t-1]
                a0 = prep.tile([128, bc], FP, tag="A0")   # nrT[t]
                ap_ = prep.tile([128, bc], FP, tag="AP")  # nrT[t+1]
                if blk == 0:
                    nc.sync.dma_start(am[0:1, :], nrT[T_NR - 1:T_NR, :])
                    nc.sync.dma_start(am[1:tn, :], nrT[0:tn - 1, :])
                else:
                    nc.sync.dma_start(am[0:tn, :], nrT[t0 - 1:t0 + tn - 1, :])
                nc.sync.dma_start(a0[0:tn, :], nrT[t0:t0 + tn, :])
                nc.sync.dma_start(ap_[0:tn, :], nrT[t0 + 1:t0 + tn + 1, :])
                dt_f = prep.tile([128, bc], FP, tag="DF")
                a0r = prep.tile([128, bc], MMD, tag="A0R")
                dtr = prep.tile([128, bc], MMD, tag="D")
                kbr = prep.tile([128, bc], MMD, tag="KBR")
                p2 = prep.tile([128, bc], MMD, tag="P2")
                p3 = prep.tile([128, bc], MMD, tag="P3")
                p4 = prep.tile([128, bc], MMD, tag="P4")
                # dnr' = nr[t+1] - nr[t-1]
                nc.vector.tensor_sub(dt_f[0:tn, :], ap_[0:tn, :], am[0:tn, :])
                nc.vector.tensor_copy(a0r[0:tn, :], a0[0:tn, :])
                nc.vector.tensor_copy(dtr[0:tn, :], dt_f[0:tn, :])
                nc.vector.tensor_copy(kbr[0:tn, :], kb[0:tn, :])
                nc.vector.tensor_mul(p2[0:tn, :], a0[0:tn, :], a0[0:tn, :])
                nc.vector.tensor_mul(p3[0:tn, :], kb[0:tn, :], a0[0:tn, :])
                nc.vector.tensor_mul(p4[0:tn, :], a0[0:tn, :], dt_f[0:tn, :])
                nc.sync.dma_start(fdram[t0:t0 + tn, 0, :], a0r[0:tn, :])
                nc.sync.dma_start(fdram[t0:t0 + tn, 1, :], dtr[0:tn, :])
                nc.sync.dma_start(fdram[t0:t0 + tn, 2, :], kbr[0:tn, :])
                nc.sync.dma_start(fdram[t0:t0 + tn, 3, :], p2[0:tn, :])
                nc.sync.dma_start(fdram[t0:t0 + tn, 4, :], p3[0:tn, :])
                nc.sync.dma_start(fdram[t0:t0 + tn, 5, :], p4[0:tn, :])

            # ---- phase 2: the scan ----
            msz1 = [128, 72]
            for t in range(nt_out):
                cur = nit[t % 2]
                prv = nit[(t + 1) % 2]
                curq = niq[t % 2]
                h1t = h1tp[t % 2]
                h2t = h2tp[t % 2]
                if t < 3:
                    nc.sync.dma_start(cur[0:1, :], ivT[t:t + 1, :])
                    # exact (unrounded) iv straight to the output row
                    nc.sync.dma_start(outT[t:t + 1, :],
                                      ivF[t:t + 1, :].bitcast(MMD))
                else:
                    p = 32 * (t % 3)
                    for c in range(nchunk):
                        cs = slice(c * ch, (c + 1) * ch)
                        # ni[t] = (delta + K0) + ni[t-1]
                        nc.vector.scalar_tensor_tensor(
                            cur[0:1, cs], dlt[p:p + 1, cs], k0t[0:1, :],
                            prv[0:1, cs], op0=mybir.AluOpType.add,
                            op1=mybir.AluOpType.add)
                    nc.sync.dma_start(outT[t:t + 1, :], cur[0:1, :])
                if t > nsteps - 1:
                    continue
                nc.scalar.activation(curq[0:1, :], cur[0:1, :],
                                     mybir.ActivationFunctionType.Square,
                                     bias=zb32[0:1, :])

                ft = fpool.tile([6, bc], MMD, tag="F")
                nc.sync.dma_start(ft[:], fdram[t, :, :])

                # mm1: h1 = relu(W1s.T @ [static; ni, ni^2] + b1)
                for m in range(2):
                    ms = slice(m * 128, m * 128 + msz1[m])
                    for c in range(nchunk):
                        cs = slice(c * ch, (c + 1) * ch)
                        h1p = h1pp.tile([128, ch], FP, tag="h1p")
                        nc.tensor.matmul(h1p[0:msz1[m], :], w1s[:, ms],
                                         ft[:, cs], start=True, stop=False)
                        if "dyn" in DBG_SKIP:
                            nc.tensor.matmul(h1p[0:msz1[m], :],
                                             w1s[0:1, ms], ft[0:1, cs],
                                             start=False, stop=True)
                        else:
                            nc.tensor.matmul(h1p[0:msz1[m], :],
                                             w1dn[0:1, ms],
                                             cur[0:1, cs], start=False,
                                             stop=False)
                            nc.tensor.matmul(h1p[0:msz1[m], :],
                                             w1dq[0:1, ms],
                                             curq[0:1, cs], start=False,
                                             stop=True)
                        bb = b1a if m == 0 else b1b
                        if (m + c) % 2 == 0:
                            nc.scalar.activation(
                                h1t[m][:, cs], h1p[0:msz1[m], :],
                                mybir.ActivationFunctionType.Relu,
                                bias=bb[:])
                        else:
                            nc.vector.scalar_tensor_tensor(
                                h1t[m][:, cs], h1p[0:msz1[m], :], bb[:],
                                zer[0:msz1[m], :],
                                op0=mybir.AluOpType.add,
                                op1=mybir.AluOpType.max)

                # mm2: h2 = relu(W2.T @ h1 + b2)
                for m in range(2):
                    ms = slice(m * 128, m * 128 + msz1[m])
                    for c in range(nchunk):
                        cs = slice(c * ch, (c + 1) * ch)
                        h2p = h2pp.tile([128, ch], FP, tag="h2p")
                        nc.tensor.matmul(h2p[0:msz1[m], :], w2k0[:, ms],
                                         h1t[0][:, cs], start=True,
                                         stop=False)
                        nc.tensor.matmul(h2p[0:msz1[m], :], w2k1[:, ms],
                                         h1t[1][:, cs], start=False,
                                         stop=True)
                        bb = b2a if m == 0 else b2b
                        if (m + c) % 2 == 1:
                            nc.scalar.activation(
                                h2t[m][0:msz1[m], cs], h2p[0:msz1[m], :],
                                mybir.ActivationFunctionType.Relu,
                                bias=bb[:])
                        else:
                            nc.vector.scalar_tensor_tensor(
                                h2t[m][0:msz1[m], cs], h2p[0:msz1[m], :], bb[:],
                                zer[0:msz1[m], :],
                                op0=mybir.AluOpType.add,
                                op1=mybir.AluOpType.max)

                # mm3: accumulate a_s * g(t) (+K0 via ones row) into the
                # 3 delta phase rows.  j: 0 = a0(start, +K0), 1 = a1, 2 = a2(stop)
                writes = []
                if t <= nsteps - 3:
                    writes.append((0, t % 3, True, False))
                if t >= (t + 2) % 3 and t <= nsteps - 2:
                    writes.append((1, (t + 2) % 3, False, False))
                if t >= (t + 1) % 3:
                    writes.append((2, (t + 1) % 3, False, True))
                if "dlt" in DBG_SKIP:
                    writes = [w_ for w_ in writes if w_[2]]  # start-only
                for (j, p, st_, sp_) in writes:
                    pp = 0 if "dlt" in DBG_SKIP else 32 * p
                    for c in range(nchunk):
                        cs = slice(c * ch, (c + 1) * ch)
                        nc.tensor.matmul(dlt[pp:pp + 1, cs], ak0[:, j:j + 1],
                                         h2t[0][:, cs], start=st_,
                                         stop=False)
                        nc.tensor.matmul(dlt[pp:pp + 1, cs], ak1[:, j:j + 1],
                                         h2t[1][:, cs], start=False,
                                         stop=sp_)
    nc.compile()
    return nc


def _to_dlt(x):
    if USE_F32R:
        import ml_dtypes
        return np.ascontiguousarray(np.asarray(x, np.float32)).astype(
            ml_dtypes.bfloat16)
    return np.ascontiguousarray(np.asarray(x, np.float32))


def round_f32r(x):
    """Round fp32 to the FP32R grid (e8m11): RNE at mantissa bit 12."""
    if not USE_F32R:
        return np.ascontiguousarray(np.asarray(x, np.float32))
    b = np.ascontiguousarray(np.asarray(x, np.float32)).view(np.uint32)
    lsb = (b >> 12) & 1
    b = b + 0x7FF + lsb
    b &= np.uint32(0xFFFFF000)
    return b.view(np.float32)


def prep_host_inputs(nr, iv, k, W1, b1, W2, b2, W3, b3, Ws, bs):
    """Shared (replicated) weight-derived arrays."""
    W1 = np.asarray(W1, np.float32)
    perm = [0, 2, 3, 4, 6, 7, 1, 5]
    scale = np.array([1, 1 / (2 * H), 1, 1, 2, 1 / (2 * H), 1, 1], np.float32)
    W1s = (W1[perm, :] * scale[:, None]).astype(np.float32)
    a = (H * np.asarray(Ws, np.float64)).astype(np.float32)
    K0 = np.float32(H * (np.float64(b3[0]) * np.asarray(Ws, np.float64).sum()
                         + np.float64(bs[0])))
    W3c = np.asarray(W3, np.float32)[:, 0]
    ak0 = np.stack([a[j] * W3c[0:128] for j in range(3)], 1).astype(np.float32)
    ak1 = np.zeros((72, 3), np.float32)
    for j in range(3):
        ak1[:, j] = a[j] * W3c[128:200]
    # merged stale-pair lhsT: variant r = t%3 writes a2 into phase (r+1)%3
    # and a1 into phase (r+2)%3 (phase p at partition 32p of dlt)
    smv = {}
    for r in range(3):
        p1, p2 = 32 * ((r + 1) % 3), 32 * ((r + 2) % 3)
        base = min(p1, p2)
        mm_ = max(p1, p2) - base + 1
        for k, (lo, hi) in enumerate(((0, 128), (128, 200))):
            arr = np.zeros((hi - lo, mm_), np.float32)
            arr[:, p1 - base] = a[2] * W3c[lo:hi]
            arr[:, p2 - base] = a[1] * W3c[lo:hi]
            smv[f"SM{r}k{k}"] = _to_dlt(arr)
    b1 = np.asarray(b1, np.float32)
    b2 = np.asarray(b2, np.float32)
    return {
        "W1stat": round_f32r(W1s[0:6]),
        "W1dni": round_f32r(W1s[6:7]),
        "W1dni2": round_f32r(W1s[7:8]),
        "W2k0": round_f32r(np.asarray(W2, np.float32)[0:128]),
        "W2k1": round_f32r(np.asarray(W2, np.float32)[128:200]),
        "b1a": np.ascontiguousarray(b1[0:128, None]),
        "b1b": np.ascontiguousarray(b1[128:200, None]),
        "b2a": np.ascontiguousarray(b2[0:128, None]),
        "b2b": np.ascontiguousarray(b2[128:200, None]),
        "Ak0": _to_dlt(ak0),
        "Ak1": _to_dlt(ak1),
        "K0v": np.full((1, 1), K0, np.float32),
        **smv,
    }


_CACHED_NC = None


def _get_nc():
    global _CACHED_NC
    if _CACHED_NC is None:
        _CACHED_NC = build_program()
    return _CACHED_NC


def run(inputs, trace=False):
    from concourse.bass_utils import run_bass_kernel_spmd

    nr = np.asarray(inputs["nr"], np.float32)
    iv = np.asarray(inputs["iv"], np.float32)
    k = np.asarray(inputs["k"], np.float32)
    assert int(inputs["stop"]) == 512
    shared = prep_host_inputs(nr, iv, k, inputs["W1"], inputs["b1"],
                              inputs["W2"], inputs["b2"], inputs["W3"],
                              inputs["b3"], inputs["Ws"], inputs["bs"])
    in_maps = []
    for i in range(NCORES):
        sl = slice(i * BC, (i + 1) * BC)
        m = dict(shared)
        m["nrT"] = np.ascontiguousarray(nr[sl].T)
        m["ivT"] = round_f32r(iv[sl].T)
        m["ivF"] = np.ascontiguousarray(iv[sl].T)
        m["kb"] = round_f32r(np.broadcast_to(k[sl][None, :], (128, BC)))
        in_maps.append(m)

    nc = _get_nc()
    res = run_bass_kernel_spmd(nc, in_maps, list(range(NCORES)), trace=trace)
    out = np.concatenate([res.results[i]["outT"].T for i in range(NCORES)], 0)
    return out.astype(np.float32), res


def kernel(**inputs):
    out, _ = run(inputs, trace=False)
    return out


def time_hw(inputs, iters=5):
    """Time device execution with device-resident inputs (no host
    transfers in the timed region).  Returns (output, [sec_per_iter])."""
    import time
    import jax
    from jax.sharding import Mesh, PartitionSpec, NamedSharding
    from jax.experimental.shard_map import shard_map
    from concourse import bass2jax

    nr = np.asarray(inputs["nr"], np.float32)
    iv = np.asarray(inputs["iv"], np.float32)
    k = np.asarray(inputs["k"], np.float32)
    shared = prep_host_inputs(nr, iv, k, inputs["W1"], inputs["b1"],
                              inputs["W2"], inputs["b2"], inputs["W3"],
                              inputs["b3"], inputs["Ws"], inputs["bs"])
    in_maps = []
    for i in range(NCORES):
        sl = slice(i * BC, (i + 1) * BC)
        m = dict(shared)
        m["nrT"] = np.ascontiguousarray(nr[sl].T)
        m["ivT"] = round_f32r(iv[sl].T)
        m["ivF"] = np.ascontiguousarray(iv[sl].T)
        m["kb"] = round_f32r(np.broadcast_to(k[sl][None, :], (128, BC)))
        in_maps.append(m)

    nc = _get_nc()
    bass2jax.install_neuronx_cc_hook()
    import concourse.mybir as mb
    pname = nc.partition_id_tensor.name if nc.partition_id_tensor else None
    in_names, out_names, out_avals, zero_outs = [], [], [], []
    for alloc in nc.m.functions[0].allocations:
        if not isinstance(mb.MemoryLocationSet, type) or not isinstance(
                alloc, mb.MemoryLocationSet):
            continue
        name = alloc.memorylocations[0].name
        if alloc.kind == "ExternalInput":
            if name != pname:
                in_names.append(name)
        elif alloc.kind == "ExternalOutput":
            out_names.append(name)
            shape = tuple(alloc.tensor_shape)
            dtype = mb.dt.np(alloc.dtype)
            out_avals.append(jax.core.ShapedArray(shape, dtype))
            zero_outs.append(np.zeros(shape, dtype))
    n_params = len(in_names)
    all_in = list(in_names) + list(out_names)
    if pname is not None:
        all_in.append(pname)

    def _body(*args):
        operands = list(args)
        if pname is not None:
            operands.append(bass2jax.partition_id_tensor())
        outs = bass2jax._bass_exec_p.bind(
            *operands,
            out_avals=tuple(out_avals),
            in_names=tuple(all_in),
            out_names=tuple(out_names),
            lowering_input_output_aliases=(),
            sim_require_finite=True,
            sim_require_nnan=True,
            nc=nc,
        )
        return tuple(outs)

    devices = jax.devices()[:NCORES]
    mesh = Mesh(np.asarray(devices), ("core",))
    specs = (PartitionSpec("core"),)
    fn = jax.jit(shard_map(_body, mesh=mesh,
                           in_specs=specs * (n_params + len(out_names)),
                           out_specs=specs * len(out_names),
                           check_rep=False))
    concat_in = [np.concatenate([np.asarray(in_maps[c][nm])
                                 for c in range(NCORES)], 0)
                 for nm in in_names]
    concat_zeros = [np.zeros((NCORES * z.shape[0], *z.shape[1:]), z.dtype)
                    for z in zero_outs]
    sh = NamedSharding(mesh, PartitionSpec("core"))
    dev_in = [jax.device_put(a, sh) for a in concat_in + concat_zeros]
    out = fn(*dev_in)
    jax.block_until_ready(out)
    times = []
    for _ in range(iters):
        t0 = time.perf_counter()
        out = fn(*dev_in)
        jax.block_until_ready(out)
        times.append(time.perf_counter() - t0)
    full = np.asarray(out[0]).reshape(NCORES, NT_OUT, BC)
    res = np.concatenate([full[c].T for c in range(NCORES)], 0)
    return res.astype(np.float32), times



# revision 16
# speedup vs baseline: 1.2858x; 1.2858x over previous
"""Trainium2 Bass kernel for the ODE-Multistep problem.

Math reformulation (exact): the reference re-evaluates the tiny MLP on a
3-step sliding window every scan iteration, so each time index t is pushed
through the MLP 3 times.  Writing g(t) = W3.T @ relu(W2.T @ relu(W1.T @
f(t))) with f(t) the 8 feature channels at time t, the scan is

    ni[j] = ni[j-1] + a2*g(j-1) + a1*g(j-2) + a0*g(j-3) + K0,   j = 3..511

with a_s = H*Ws[s], K0 = H*(b3*sum(Ws) + bs).  Each g(t) is computed once
(3x compute saving).  On device the combination a2*g(j-1)+a1*g(j-2)+
a0*g(j-3)+K0 is accumulated directly in PSUM by 3 phase-rotated
accumulator rows fed from the last matmul (W3 contraction), so the only
per-step non-matmul work on the sequential critical path is one
tensor_add (ni) and one Square activation (ni^2).

Layout: feature-on-partition ("transposed") activations, batch on the
free axis.  Per core: 1024 samples, chunks of 512 (PSUM bank width).
6 static feature channels are precomputed on device into DRAM
(Fdram[t] = [nr, nr(+1)-nr(-1), k, nr^2, k*nr, nr*(nr(+1)-nr(-1))]) and
streamed in during the scan; the 2 recurrent channels (ni, ni^2) live in
SBUF partitions 32/33 and enter the first matmul as a separate K=2
accumulating matmul (row-group 32, packable with the K=6 static matmul).

Sharding: data-parallel over batch, 8 cores x 1024 samples, weights
replicated, no cross-core traffic.
"""

import os
import sys
import numpy as np

sys.path.insert(0, "/opt/trn_rl_repo")

import concourse.bass as bass
import concourse.bacc as bacc
import concourse.mybir as mybir
from concourse import tile

FP = mybir.dt.float32
H = 1e-3
B_TOT = 8192
NCORES = 8
BC = B_TOT // NCORES          # 1024 samples per core
T_NR = 513
NT_OUT = 512                  # output time steps (cols of output)
CH = 512                      # psum chunk (free) size
NCHUNK = BC // CH

# float32r: hardware single-pass fp32 matmul mode, 4x faster than fp32
USE_F32R = True
DBG_SKIP = set(os.environ.get("KERNEL_DBG_SKIP", "").split(","))


def build_program(nt_out=NT_OUT, bc=BC, f32r=None):
    """Build the single-core SPMD Bass program.

    nt_out/bc reducible for simulator smoke tests.
    """
    if f32r is None:
        f32r = USE_F32R
    MMD = mybir.dt.float32r if f32r else FP
    # delta stage runs bf16: f32r matmuls cannot write PSUM at base
    # partition 32/64 (walrus ISA check), bf16 can; cost is N-cycles either
    # way and the W3 contraction is precision-insensitive.
    DLT = mybir.dt.bfloat16 if f32r else FP

    nsteps = nt_out - 1           # g(t) computed for t = 0..nt_out-2
    nchunk = bc // CH if bc >= CH else 1
    ch = CH if bc >= CH else bc

    nc = bacc.Bacc()

    # ---- I/O ----
    nrT = nc.declare_dram_parameter("nrT", [T_NR, bc], FP, isOutput=False)
    ivT = nc.declare_dram_parameter("ivT", [3, bc], MMD, isOutput=False)
    ivF = nc.declare_dram_parameter("ivF", [3, bc], FP, isOutput=False)
    kbd = nc.declare_dram_parameter("kb", [128, bc], FP, isOutput=False)
    w1s_d = nc.declare_dram_parameter("W1stat", [6, 200], MMD, isOutput=False)
    w1dn_d = nc.declare_dram_parameter("W1dni", [1, 200], MMD, isOutput=False)
    w1dq_d = nc.declare_dram_parameter("W1dni2", [1, 200], MMD, isOutput=False)
    w2k0_d = nc.declare_dram_parameter("W2k0", [128, 200], MMD, isOutput=False)
    w2k1_d = nc.declare_dram_parameter("W2k1", [72, 200], MMD, isOutput=False)
    b1a_d = nc.declare_dram_parameter("b1a", [128, 1], FP, isOutput=False)
    b1b_d = nc.declare_dram_parameter("b1b", [72, 1], FP, isOutput=False)
    b2a_d = nc.declare_dram_parameter("b2a", [128, 1], FP, isOutput=False)
    b2b_d = nc.declare_dram_parameter("b2b", [72, 1], FP, isOutput=False)
    ak0_d = nc.declare_dram_parameter("Ak0", [128, 3], DLT, isOutput=False)
    ak1_d = nc.declare_dram_parameter("Ak1", [72, 3], DLT, isOutput=False)
    k0_d = nc.declare_dram_parameter("K0v", [1, 1], FP, isOutput=False)
    sm_d = [[nc.declare_dram_parameter(f"SM{r}k{k}", [(128, 72)[k], (33, 65, 33)[r]],
                                       DLT, isOutput=False)
             for k in range(2)] for r in range(3)]
    outT = nc.declare_dram_parameter("outT", [nt_out, bc], MMD, isOutput=True)

    fdram = nc.dram_tensor("Fdram", [nsteps, 6, bc], MMD)

    with tile.TileContext(nc) as tc:
        with (
            tc.tile_pool(name="const", bufs=1) as constp,
            tc.tile_pool(name="state", bufs=1) as statep,
            tc.tile_pool(name="fpool", bufs=6) as fpool,
            tc.tile_pool(name="pre", bufs=3) as prep,
            tc.tile_pool(name="h1psum", bufs=3, space="PSUM") as h1pp,
            tc.tile_pool(name="h2psum", bufs=3, space="PSUM") as h2pp,
            tc.tile_pool(name="dpsum", bufs=1, space="PSUM") as dpp,
        ):
            # ---- persistent SBUF ----
            w1s = constp.tile([6, 200], MMD)
            w1dn = constp.tile([1, 200], MMD)
            w1dq = constp.tile([1, 200], MMD)
            w2k0 = constp.tile([128, 200], MMD)
            w2k1 = constp.tile([72, 200], MMD)
            b1a = constp.tile([128, 1], FP)
            b1b = constp.tile([72, 1], FP)
            b2a = constp.tile([128, 1], FP)
            b2b = constp.tile([72, 1], FP)
            ak0 = constp.tile([128, 3], DLT)
            ak1 = constp.tile([72, 3], DLT)
            k0t = constp.tile([1, 1], FP)
            sm = [[constp.tile([(128, 72)[k], (33, 65, 33)[r]], DLT,
                               name=f"sm{r}k{k}", tag=f"sm{r}k{k}")
                   for k in range(2)] for r in range(3)]
            zb32 = constp.tile([1, 1], FP)
            kb = constp.tile([128, bc], FP)
            zer = constp.tile([128, ch], FP)

            nc.sync.dma_start(w1s[:], w1s_d[:])
            nc.sync.dma_start(w1dn[0:1, :], w1dn_d[:])
            nc.sync.dma_start(w1dq[0:1, :], w1dq_d[:])
            nc.sync.dma_start(w2k0[:], w2k0_d[:])
            nc.sync.dma_start(w2k1[:], w2k1_d[:])
            nc.sync.dma_start(b1a[:], b1a_d[:])
            nc.sync.dma_start(b1b[:], b1b_d[:])
            nc.sync.dma_start(b2a[:], b2a_d[:])
            nc.sync.dma_start(b2b[:], b2b_d[:])
            nc.sync.dma_start(ak0[:], ak0_d[:])
            nc.sync.dma_start(ak1[:], ak1_d[:])
            nc.sync.dma_start(k0t[0:1, :], k0_d[:])
            for r in range(3):
                for k in range(2):
                    nc.sync.dma_start(sm[r][k][:], sm_d[r][k][:])
            nc.vector.memset(zb32[:], 0.0)
            nc.sync.dma_start(kb[:], kbd[:])
            nc.vector.memset(zer[:], 0.0)

            # ---- ni state: ni at partition 32, ni^2 at partition 64 ----
            nit = [statep.tile([1, bc], MMD, name=f"nit{i}", tag=f"nit{i}")
                   for i in range(2)]
            niq = [statep.tile([1, bc], MMD, name=f"niq{i}", tag=f"niq{i}")
                   for i in range(2)]

            # ---- h1T / h2T activations (persistent, rewritten per step) ----
            h1tp = [[statep.tile([128, bc], MMD, name=f"h1t0_{i}",
                                 tag=f"h1t0_{i}"),
                     statep.tile([72, bc], MMD, name=f"h1t1_{i}",
                                 tag=f"h1t1_{i}")] for i in range(2)]
            h2tp = [[statep.tile([128, bc], DLT, name=f"h2t0_{i}",
                                 tag=f"h2t0_{i}"),
                     statep.tile([72, bc], DLT, name=f"h2t1_{i}",
                                 tag=f"h2t1_{i}")] for i in range(2)]

            # delta accumulator: 3 phase rows
            # phase p lives at partition 32*p (matmul out base must be 0/32/64)
            dlt = dpp.tile([65, nchunk * ch], FP)

            # ---- phase 1: precompute static features into Fdram ----
            # engine APs need 32-aligned base partitions, so the +-1 shifted
            # views are materialized by three separate (overlapping) DMAs.
            for blk in range((nsteps + 127) // 128):
                t0 = blk * 128
                tn = min(128, nsteps - t0)
                am = prep.tile([128, bc], FP, tag="AM")   # nrT[t-1]
                a0 = prep.tile([128, bc], FP, tag="A0")   # nrT[t]
                ap_ = prep.tile([128, bc], FP, tag="AP")  # nrT[t+1]
                if blk == 0:
                    nc.sync.dma_start(am[0:1, :], nrT[T_NR - 1:T_NR, :])
                    nc.sync.dma_start(am[1:tn, :], nrT[0:tn - 1, :])
                else:
                    nc.sync.dma_start(am[0:tn, :], nrT[t0 - 1:t0 + tn - 1, :])
                nc.sync.dma_start(a0[0:tn, :], nrT[t0:t0 + tn, :])
                nc.sync.dma_start(ap_[0:tn, :], nrT[t0 + 1:t0 + tn + 1, :])
                dt_f = prep.tile([128, bc], FP, tag="DF")
                a0r = prep.tile([128, bc], MMD, tag="A0R")
                dtr = prep.tile([128, bc], MMD, tag="D")
                kbr = prep.tile([128, bc], MMD, tag="KBR")
                p2 = prep.tile([128, bc], MMD, tag="P2")
                p3 = prep.tile([128, bc], MMD, tag="P3")
                p4 = prep.tile([128, bc], MMD, tag="P4")
                # dnr' = nr[t+1] - nr[t-1]
                nc.vector.tensor_sub(dt_f[0:tn, :], ap_[0:tn, :], am[0:tn, :])
                nc.vector.tensor_copy(a0r[0:tn, :], a0[0:tn, :])
                nc.vector.tensor_copy(dtr[0:tn, :], dt_f[0:tn, :])
                nc.vector.tensor_copy(kbr[0:tn, :], kb[0:tn, :])
                nc.vector.tensor_mul(p2[0:tn, :], a0[0:tn, :], a0[0:tn, :])
                nc.vector.tensor_mul(p3[0:tn, :], kb[0:tn, :], a0[0:tn, :])
                nc.vector.tensor_mul(p4[0:tn, :], a0[0:tn, :], dt_f[0:tn, :])
                nc.sync.dma_start(fdram[t0:t0 + tn, 0, :], a0r[0:tn, :])
                nc.sync.dma_start(fdram[t0:t0 + tn, 1, :], dtr[0:tn, :])
                nc.sync.dma_start(fdram[t0:t0 + tn, 2, :], kbr[0:tn, :])
                nc.sync.dma_start(fdram[t0:t0 + tn, 3, :], p2[0:tn, :])
                nc.sync.dma_start(fdram[t0:t0 + tn, 4, :], p3[0:tn, :])
                nc.sync.dma_start(fdram[t0:t0 + tn, 5, :], p4[0:tn, :])

            # ---- phase 2: the scan ----
            msz1 = [128, 72]
            for t in range(nt_out):
                cur = nit[t % 2]
                prv = nit[(t + 1) % 2]
                curq = niq[t % 2]
                h1t = h1tp[t % 2]
                h2t = h2tp[t % 2]
                if t < 3:
                    nc.sync.dma_start(cur[0:1, :], ivT[t:t + 1, :])
                    # exact (unrounded) iv straight to the output row
                    nc.sync.dma_start(outT[t:t + 1, :],
                                      ivF[t:t + 1, :].bitcast(MMD))
                else:
                    p = 32 * (t % 3)
                    for c in range(nchunk):
                        cs = slice(c * ch, (c + 1) * ch)
                        # ni[t] = (delta + K0) + ni[t-1]
                        nc.vector.scalar_tensor_tensor(
                            cur[0:1, cs], dlt[p:p + 1, cs], k0t[0:1, :],
                            prv[0:1, cs], op0=mybir.AluOpType.add,
                            op1=mybir.AluOpType.add)
                    nc.sync.dma_start(outT[t:t + 1, :], cur[0:1, :])
                if t > nsteps - 1:
                    continue
                nc.scalar.activation(curq[0:1, :], cur[0:1, :],
                                     mybir.ActivationFunctionType.Square,
                                     bias=zb32[0:1, :])

                ft = fpool.tile([6, bc], MMD, tag="F")
                nc.sync.dma_start(ft[:], fdram[t, :, :])

                # mm1: h1 = relu(W1s.T @ [static; ni, ni^2] + b1)
                for m in range(2):
                    ms = slice(m * 128, m * 128 + msz1[m])
                    for c in range(nchunk):
                        cs = slice(c * ch, (c + 1) * ch)
                        h1p = h1pp.tile([128, ch], FP, tag="h1p")
                        nc.tensor.matmul(h1p[0:msz1[m], :], w1s[:, ms],
                                         ft[:, cs], start=True, stop=False)
                        if "dyn" in DBG_SKIP:
                            nc.tensor.matmul(h1p[0:msz1[m], :],
                                             w1s[0:1, ms], ft[0:1, cs],
                                             start=False, stop=True)
                        else:
                            nc.tensor.matmul(h1p[0:msz1[m], :],
                                             w1dn[0:1, ms],
                                             cur[0:1, cs], start=False,
                                             stop=False)
                            nc.tensor.matmul(h1p[0:msz1[m], :],
                                             w1dq[0:1, ms],
                                             curq[0:1, cs], start=False,
                                             stop=True)
                        bb = b1a if m == 0 else b1b
                        if (m + c) % 2 == 0:
                            nc.scalar.activation(
                                h1t[m][:, cs], h1p[0:msz1[m], :],
                                mybir.ActivationFunctionType.Relu,
                                bias=bb[:])
                        else:
                            nc.vector.scalar_tensor_tensor(
                                h1t[m][:, cs], h1p[0:msz1[m], :], bb[:],
                                zer[0:msz1[m], :],
                                op0=mybir.AluOpType.add,
                                op1=mybir.AluOpType.max)

                # mm2: h2 = relu(W2.T @ h1 + b2)
                for m in range(2):
                    ms = slice(m * 128, m * 128 + msz1[m])
                    for c in range(nchunk):
                        cs = slice(c * ch, (c + 1) * ch)
                        h2p = h2pp.tile([128, ch], FP, tag="h2p")
                        nc.tensor.matmul(h2p[0:msz1[m], :], w2k0[:, ms],
                                         h1t[0][:, cs], start=True,
                                         stop=False)
                        nc.tensor.matmul(h2p[0:msz1[m], :], w2k1[:, ms],
                                         h1t[1][:, cs], start=False,
                                         stop=True)
                        bb = b2a if m == 0 else b2b
                        if (m + c) % 2 == 1:
                            nc.scalar.activation(
                                h2t[m][0:msz1[m], cs], h2p[0:msz1[m], :],
                                mybir.ActivationFunctionType.Relu,
                                bias=bb[:])
                        else:
                            nc.vector.scalar_tensor_tensor(
                                h2t[m][0:msz1[m], cs], h2p[0:msz1[m], :], bb[:],
                                zer[0:msz1[m], :],
                                op0=mybir.AluOpType.add,
                                op1=mybir.AluOpType.max)

                # mm3: accumulate a_s * g(t) (+K0 via ones row) into the
                # 3 delta phase rows.  j: 0 = a0(start, +K0), 1 = a1, 2 = a2(stop)
                writes = []
                if t <= nsteps - 3:
                    writes.append((0, t % 3, True, False))
                if t >= (t + 2) % 3 and t <= nsteps - 2:
                    writes.append((1, (t + 2) % 3, False, False))
                if t >= (t + 1) % 3:
                    writes.append((2, (t + 1) % 3, False, True))
                if "dlt" in DBG_SKIP:
                    writes = [w_ for w_ in writes if w_[2]]  # start-only
                for (j, p, st_, sp_) in writes:
                    pp = 0 if "dlt" in DBG_SKIP else 32 * p
                    for c in range(nchunk):
                        cs = slice(c * ch, (c + 1) * ch)
                        nc.tensor.matmul(dlt[pp:pp + 1, cs], ak0[:, j:j + 1],
                                         h2t[0][:, cs], start=st_,
                                         stop=False)
                        nc.tensor.matmul(dlt[pp:pp + 1, cs], ak1[:, j:j + 1],
                                         h2t[1][:, cs], start=False,
                                         stop=sp_)
    nc.compile()
    return nc


def _to_dlt(x):
    if USE_F32R:
        import ml_dtypes
        return np.ascontiguousarray(np.asarray(x, np.float32)).astype(
            ml_dtypes.bfloat16)
    return np.ascontiguousarray(np.asarray(x, np.float32))


def round_f32r(x):
    """Round fp32 to the FP32R grid (e8m11): RNE at mantissa bit 12."""
    if not USE_F32R:
        return np.ascontiguousarray(np.asarray(x, np.float32))
    b = np.ascontiguousarray(np.asarray(x, np.float32)).view(np.uint32)
    lsb = (b >> 12) & 1
    b = b + 0x7FF + lsb
    b &= np.uint32(0xFFFFF000)
    return b.view(np.float32)


def prep_host_inputs(nr, iv, k, W1, b1, W2, b2, W3, b3, Ws, bs):
    """Shared (replicated) weight-derived arrays."""
    W1 = np.asarray(W1, np.float32)
    perm = [0, 2, 3, 4, 6, 7, 1, 5]
    scale = np.array([1, 1 / (2 * H), 1, 1, 2, 1 / (2 * H), 1, 1], np.float32)
    W1s = (W1[perm, :] * scale[:, None]).astype(np.float32)
    a = (H * np.asarray(Ws, np.float64)).astype(np.float32)
    K0 = np.float32(H * (np.float64(b3[0]) * np.asarray(Ws, np.float64).sum()
                         + np.float64(bs[0])))
    W3c = np.asarray(W3, np.float32)[:, 0]
    ak0 = np.stack([a[j] * W3c[0:128] for j in range(3)], 1).astype(np.float32)
    ak1 = np.zeros((72, 3), np.float32)
    for j in range(3):
        ak1[:, j] = a[j] * W3c[128:200]
    # merged stale-pair lhsT: variant r = t%3 writes a2 into phase (r+1)%3
    # and a1 into phase (r+2)%3 (phase p at partition 32p of dlt)
    smv = {}
    for r in range(3):
        p1, p2 = 32 * ((r + 1) % 3), 32 * ((r + 2) % 3)
        base = min(p1, p2)
        mm_ = max(p1, p2) - base + 1
        for k, (lo, hi) in enumerate(((0, 128), (128, 200))):
            arr = np.zeros((hi - lo, mm_), np.float32)
            arr[:, p1 - base] = a[2] * W3c[lo:hi]
            arr[:, p2 - base] = a[1] * W3c[lo:hi]
            smv[f"SM{r}k{k}"] = _to_dlt(arr)
    b1 = np.asarray(b1, np.float32)
    b2 = np.asarray(b2, np.float32)
    return {
        "W1stat": round_f32r(W1s[0:6]),
        "W1dni": round_f32r(W1s[6:7]),
        "W1dni2": round_f32r(W1s[7:8]),
        "W2k0": round_f32r(np.asarray(W2, np.float32)[0:128]),
        "W2k1": round_f32r(np.asarray(W2, np.float32)[128:200]),
        "b1a": np.ascontiguousarray(b1[0:128, None]),
        "b1b": np.ascontiguousarray(b1[128:200, None]),
        "b2a": np.ascontiguousarray(b2[0:128, None]),
        "b2b": np.ascontiguousarray(b2[128:200, None]),
        "Ak0": _to_dlt(ak0),
        "Ak1": _to_dlt(ak1),
        "K0v": np.full((1, 1), K0, np.float32),
        **smv,
    }


_CACHED_NC = None


def _get_nc():
    global _CACHED_NC
    if _CACHED_NC is None:
        _CACHED_NC = build_program()
    return _CACHED_NC


def run(inputs, trace=False):
    from concourse.bass_utils import run_bass_kernel_spmd

    nr = np.asarray(inputs["nr"], np.float32)
    iv = np.asarray(inputs["iv"], np.float32)
    k = np.asarray(inputs["k"], np.float32)
    assert int(inputs["stop"]) == 512
    shared = prep_host_inputs(nr, iv, k, inputs["W1"], inputs["b1"],
                              inputs["W2"], inputs["b2"], inputs["W3"],
                              inputs["b3"], inputs["Ws"], inputs["bs"])
    in_maps = []
    for i in range(NCORES):
        sl = slice(i * BC, (i + 1) * BC)
        m = dict(shared)
        m["nrT"] = np.ascontiguousarray(nr[sl].T)
        m["ivT"] = round_f32r(iv[sl].T)
        m["ivF"] = np.ascontiguousarray(iv[sl].T)
        m["kb"] = round_f32r(np.broadcast_to(k[sl][None, :], (128, BC)))
        in_maps.append(m)

    nc = _get_nc()
    res = run_bass_kernel_spmd(nc, in_maps, list(range(NCORES)), trace=trace)
    out = np.concatenate([res.results[i]["outT"].T for i in range(NCORES)], 0)
    return out.astype(np.float32), res


def kernel(**inputs):
    out, _ = run(inputs, trace=False)
    return out


def time_hw(inputs, iters=5):
    """Time device execution with device-resident inputs (no host
    transfers in the timed region).  Returns (output, [sec_per_iter])."""
    import time
    import jax
    from jax.sharding import Mesh, PartitionSpec, NamedSharding
    from jax.experimental.shard_map import shard_map
    from concourse import bass2jax

    nr = np.asarray(inputs["nr"], np.float32)
    iv = np.asarray(inputs["iv"], np.float32)
    k = np.asarray(inputs["k"], np.float32)
    shared = prep_host_inputs(nr, iv, k, inputs["W1"], inputs["b1"],
                              inputs["W2"], inputs["b2"], inputs["W3"],
                              inputs["b3"], inputs["Ws"], inputs["bs"])
    in_maps = []
    for i in range(NCORES):
        sl = slice(i * BC, (i + 1) * BC)
        m = dict(shared)
        m["nrT"] = np.ascontiguousarray(nr[sl].T)
        m["ivT"] = round_f32r(iv[sl].T)
        m["ivF"] = np.ascontiguousarray(iv[sl].T)
        m["kb"] = round_f32r(np.broadcast_to(k[sl][None, :], (128, BC)))
        in_maps.append(m)

    nc = _get_nc()
    bass2jax.install_neuronx_cc_hook()
    import concourse.mybir as mb
    pname = nc.partition_id_tensor.name if nc.partition_id_tensor else None
    in_names, out_names, out_avals, zero_outs = [], [], [], []
    for alloc in nc.m.functions[0].allocations:
        if not isinstance(mb.MemoryLocationSet, type) or not isinstance(
                alloc, mb.MemoryLocationSet):
            continue
        name = alloc.memorylocations[0].name
        if alloc.kind == "ExternalInput":
            if name != pname:
                in_names.append(name)
        elif alloc.kind == "ExternalOutput":
            out_names.append(name)
            shape = tuple(alloc.tensor_shape)
            dtype = mb.dt.np(alloc.dtype)
            out_avals.append(jax.core.ShapedArray(shape, dtype))
            zero_outs.append(np.zeros(shape, dtype))
    n_params = len(in_names)
    all_in = list(in_names) + list(out_names)
    if pname is not None:
        all_in.append(pname)

    def _body(*args):
        operands = list(args)
        if pname is not None:
            operands.append(bass2jax.partition_id_tensor())
        outs = bass2jax._bass_exec_p.bind(
            *operands,
            out_avals=tuple(out_avals),
            in_names=tuple(all_in),
            out_names=tuple(out_names),
            lowering_input_output_aliases=(),
            sim_require_finite=True,
            sim_require_nnan=True,
            nc=nc,
        )
        return tuple(outs)

    devices = jax.devices()[:NCORES]
    mesh = Mesh(np.asarray(devices), ("core",))
    specs = (PartitionSpec("core"),)
    fn = jax.jit(shard_map(_body, mesh=mesh,
                           in_specs=specs * (n_params + len(out_names)),
                           out_specs=specs * len(out_names),
                           check_rep=False))
    concat_in = [np.concatenate([np.asarray(in_maps[c][nm])
                                 for c in range(NCORES)], 0)
                 for nm in in_names]
    concat_zeros = [np.zeros((NCORES * z.shape[0], *z.shape[1:]), z.dtype)
                    for z in zero_outs]
    sh = NamedSharding(mesh, PartitionSpec("core"))
    dev_in = [jax.device_put(a, sh) for a in concat_in + concat_zeros]
    out = fn(*dev_in)
    jax.block_until_ready(out)
    times = []
    for _ in range(iters):
        t0 = time.perf_counter()
        out = fn(*dev_in)
        jax.block_until_ready(out)
        times.append(time.perf_counter() - t0)
    full = np.asarray(out[0]).reshape(NCORES, NT_OUT, BC)
    res = np.concatenate([full[c].T for c in range(NCORES)], 0)
    return res.astype(np.float32), times



# revision 17
# speedup vs baseline: 2.2023x; 1.7128x over previous
"""Trainium2 Bass kernel for the ODE-Multistep problem (fp8-DoubleRow version).

Math (exact reformulation): g(t) = W3.T relu(W2.T relu(W1s.T f(t))),
  ni[j] = ni[j-1] + a2 g(j-1) + a1 g(j-2) + a0 g(j-3) + K0.

All three matmul stages run in fp8e4 with DoubleRow perf mode (2 K-slots per
partition-row, half PE cost per output column):
  - mm1: static feature slots precomputed to DRAM as fp8 pairs; the two
    1/(2H)-scaled channels (dnr, nr*dnr) get weight-hi/lo and feature-hi/lo
    splits (3 slots each) keeping the product error ~2^-8.  b1 via a
    (b1, ones) slot.  The recurrent pair (ni*8, ni^2) is a separate
    1-partition DR matmul fed by per-step cast + square elementwise ops.
  - mm2: K=200 as 100 DR pairs (k, k+100) + (b2, ones) row; W2 split into
    hi+lo accumulating DR matmuls; h1 stored fp8 with per-hidden pow2
    scales applied by the relu (per-partition scale), undone in W2 rows.
  - mm3: same DR layout, columns a_j*W3/s2/C3, K0 on the j=0 ones row;
    accumulated into 3 phase-rotated PSUM rows.
Pow2 scaling is exact in fp8; PSUM accumulation is fp32; the recurrence
state ni stays fp32.

Sharding: data-parallel over batch, 8 cores x 1024 samples, weights
replicated, no cross-core traffic.
"""

import sys
import numpy as np

sys.path.insert(0, "/opt/trn_rl_repo")

import concourse.bass as bass
import concourse.bacc as bacc
import concourse.mybir as mybir
from concourse import tile

FP = mybir.dt.float32
F8 = mybir.dt.float8e4
DR = mybir.MatmulPerfMode.DoubleRow
H = 1e-3
B_TOT = 8192
NCORES = 8
BC = B_TOT // NCORES
T_NR = 513
NT_OUT = 512
CH = 512

S_NI = 8.0          # ni feature scale (|ni| <= 56 safe)
C3 = 2.0 ** -14     # dlt psum scale
NKI1 = 6            # mm1 static DR pair-rows
MPAD = 112          # M-stride padding for lhsT pair dim

AluOp = mybir.AluOpType
ACTF = mybir.ActivationFunctionType


def build_program(nt_out=NT_OUT, bc=BC):
    nsteps = nt_out - 1
    nchunk = max(1, bc // CH)
    ch = CH if bc >= CH else bc
    hc = ch // 2

    nc = bacc.Bacc()

    # ---- I/O ----
    nrT = nc.declare_dram_parameter("nrT", [T_NR, bc], FP, isOutput=False)
    ivT = nc.declare_dram_parameter("ivT", [3, bc], FP, isOutput=False)
    kbd = nc.declare_dram_parameter("kb", [128, bc], FP, isOutput=False)
    w1p_d = nc.declare_dram_parameter("W1P", [NKI1, 2, 2 * MPAD], F8, isOutput=False)
    w1dyn_d = nc.declare_dram_parameter("W1DYN", [1, 2, 2 * MPAD], F8, isOutput=False)
    w2hi_d = nc.declare_dram_parameter("W2HI", [101, 2, 2 * MPAD], F8, isOutput=False)
    w2lo_d = nc.declare_dram_parameter("W2LO", [101, 2, 2 * MPAD], F8, isOutput=False)
    akhi_d = nc.declare_dram_parameter("AKHI", [101, 2, 16], F8, isOutput=False)
    aklo_d = nc.declare_dram_parameter("AKLO", [101, 2, 16], F8, isOutput=False)
    s1_d = nc.declare_dram_parameter("S1", [100, 2], FP, isOutput=False)
    s2_d = nc.declare_dram_parameter("S2", [100, 2], FP, isOutput=False)
    one8_d = nc.declare_dram_parameter("ONE8D", [1, 2, CH], F8, isOutput=False)
    c3_d = nc.declare_dram_parameter("C3V", [1, 1], FP, isOutput=False)
    outT = nc.declare_dram_parameter("outT", [nt_out, bc], FP, isOutput=True)

    fdram = nc.dram_tensor("Fdram", [nsteps, NKI1, 2, bc], F8)

    with tile.TileContext(nc) as tc:
        with (
            tc.tile_pool(name="const", bufs=1) as constp,
            tc.tile_pool(name="state", bufs=1) as statep,
            tc.tile_pool(name="fpool", bufs=4) as fpool,
            tc.tile_pool(name="pre", bufs=2) as prep,
            tc.tile_pool(name="hps", bufs=6, space="PSUM") as hpp,
            tc.tile_pool(name="dps", bufs=1, space="PSUM") as dpp,
        ):
            # ---- persistent SBUF constants ----
            w1p = constp.tile([NKI1, 2, 2 * MPAD], F8)
            w1dyn = constp.tile([1, 2, 2 * MPAD], F8)
            w2hi = constp.tile([101, 2, 2 * MPAD], F8)
            w2lo = constp.tile([101, 2, 2 * MPAD], F8)
            akhi = constp.tile([101, 2, 16], F8)
            aklo = constp.tile([101, 2, 16], F8)
            s1t = constp.tile([100, 2], FP)
            s2t = constp.tile([100, 2], FP)
            c3t = constp.tile([1, 1], FP)
            kb = constp.tile([128, bc], FP)

            nc.sync.dma_start(w1p[:], w1p_d[:])
            nc.sync.dma_start(w1dyn[:], w1dyn_d[:])
            nc.sync.dma_start(w2hi[:], w2hi_d[:])
            nc.sync.dma_start(w2lo[:], w2lo_d[:])
            nc.sync.dma_start(akhi[:], akhi_d[:])
            nc.sync.dma_start(aklo[:], aklo_d[:])
            nc.sync.dma_start(s1t[:], s1_d[:])
            nc.sync.dma_start(s2t[:], s2_d[:])
            nc.sync.dma_start(c3t[:], c3_d[:])
            nc.sync.dma_start(kb[:], kbd[:])

            # ---- state ----
            nit = [statep.tile([1, bc], FP, name=f"nit{i}", tag=f"nit{i}")
                   for i in range(2)]
            dynF = [statep.tile([1, 2 * nchunk, ch], F8, name=f"dynF{i}",
                                tag=f"dynF{i}") for i in range(2)]
            h1t = [[statep.tile([101, 2, ch], F8, name=f"h1t{c}_{i}",
                                tag=f"h1t{c}_{i}") for i in range(2)]
                   for c in range(nchunk)]
            h2t = [[statep.tile([101, 2, ch], F8, name=f"h2t{c}_{i}",
                                tag=f"h2t{c}_{i}") for i in range(3)]
                   for c in range(nchunk)]
            for c in range(nchunk):
                for i in range(2):
                    nc.sync.dma_start(h1t[c][i][100:101, :, :],
                                      one8_d[:, :, 0:ch])
                for i in range(3):
                    nc.sync.dma_start(h2t[c][i][100:101, :, :],
                                      one8_d[:, :, 0:ch])

            racc = dpp.tile([1, nchunk * ch], FP)
            iv2 = statep.tile([1, bc], FP, name="iv2", tag="iv2")
            nc.sync.dma_start(iv2[0:1, :], ivT[2:3, :])

            # ---- phase 1: static feature slots -> Fdram (fp8) ----
            # slot pairs (ki: blk0, blk1): 0:(f1hi, f1lo16) 1:(f1hi, f5hi)
            #   2:(f5lo16, f5hi) 3:(f0, f2) 4:(f3, f4) 5:(ones, zero)
            for blk in range((nsteps + 127) // 128):
                t0 = blk * 128
                tn = min(128, nsteps - t0)
                am = prep.tile([128, bc], FP, tag="AM")
                a0 = prep.tile([128, bc], FP, tag="A0")
                ap_ = prep.tile([128, bc], FP, tag="AP")
                if blk == 0:
                    nc.sync.dma_start(am[0:1, :], nrT[T_NR - 1:T_NR, :])
                    nc.sync.dma_start(am[1:tn, :], nrT[0:tn - 1, :])
                else:
                    nc.sync.dma_start(am[0:tn, :], nrT[t0 - 1:t0 + tn - 1, :])
                nc.sync.dma_start(a0[0:tn, :], nrT[t0:t0 + tn, :])
                nc.sync.dma_start(ap_[0:tn, :], nrT[t0 + 1:t0 + tn + 1, :])
                dnr = prep.tile([128, bc], FP, tag="DNR")
                tmp = prep.tile([128, bc], FP, tag="TMP")
                p5 = prep.tile([128, bc], FP, tag="P5")
                f1hi = prep.tile([128, bc], F8, tag="F1HI")
                f1lo = prep.tile([128, bc], F8, tag="F1LO")
                f5hi = prep.tile([128, bc], F8, tag="F5HI")
                f5lo = prep.tile([128, bc], F8, tag="F5LO")
                f0 = prep.tile([128, bc], F8, tag="F0")
                f2 = prep.tile([128, bc], F8, tag="F2")
                f3 = prep.tile([128, bc], F8, tag="F3")
                f4 = prep.tile([128, bc], F8, tag="F4")
                one8 = prep.tile([128, bc], F8, tag="ONE8")
                zz8 = prep.tile([128, bc], F8, tag="ZZ8")
                s = slice(0, tn)
                # channels 1/5 carry x16 (their W rows carry /16 to fit fp8)
                nc.vector.tensor_sub(dnr[s, :], ap_[s, :], am[s, :])
                nc.vector.tensor_scalar_mul(f1hi[s, :], dnr[s, :], 16.0)
                nc.vector.scalar_tensor_tensor(
                    tmp[s, :], dnr[s, :], 16.0, f1hi[s, :],
                    op0=AluOp.mult, op1=AluOp.subtract)
                nc.vector.tensor_scalar_mul(f1lo[s, :], tmp[s, :], 16.0)
                nc.vector.tensor_mul(p5[s, :], a0[s, :], dnr[s, :])
                nc.vector.tensor_scalar_mul(f5hi[s, :], p5[s, :], 8.0)
                nc.vector.scalar_tensor_tensor(
                    tmp[s, :], p5[s, :], 8.0, f5hi[s, :],
                    op0=AluOp.mult, op1=AluOp.subtract)
                nc.vector.tensor_scalar_mul(f5lo[s, :], tmp[s, :], 16.0)
                nc.vector.tensor_copy(f0[s, :], a0[s, :])
                nc.vector.tensor_copy(f2[s, :], kb[s, :])
                nc.vector.scalar_tensor_tensor(
                    f3[s, :], a0[s, :], 1.0, a0[s, :],
                    op0=AluOp.mult, op1=AluOp.mult)
                nc.vector.scalar_tensor_tensor(
                    f4[s, :], kb[s, :], 1.0, a0[s, :],
                    op0=AluOp.mult, op1=AluOp.mult)
                nc.vector.memset(one8[s, :], 1.0)
                nc.vector.memset(zz8[s, :], 0.0)
                ts_ = slice(t0, t0 + tn)
                for (ki, b_, src) in [(0, 0, f1hi), (0, 1, f1lo), (1, 0, f1hi),
                                      (1, 1, f5hi), (2, 0, f5lo), (2, 1, f5hi),
                                      (3, 0, f0), (3, 1, f2), (4, 0, f3),
                                      (4, 1, f4), (5, 0, one8), (5, 1, zz8)]:
                    nc.sync.dma_start(fdram[ts_, ki, b_, :], src[s, :])

            # ---- phase 2: scan ----
            def relu_op(eng, out_ap, psum_ap, scale_ap, np_):
                if eng == "act":
                    nc.scalar.activation(out_ap, psum_ap, ACTF.Relu,
                                         scale=scale_ap)
                elif eng == "dve":
                    nc.vector.tensor_scalar(out_ap, psum_ap, scale_ap, 0.0,
                                            op0=AluOp.mult, op1=AluOp.max)
                else:
                    nc.gpsimd.tensor_scalar(out_ap, psum_ap, scale_ap, 0.0,
                                            op0=AluOp.mult, op1=AluOp.max)

            RELU1_ENG = [["act", "dve"], ["act", "dve"]]
            RELU2_ENG = [["dve", "act"], ["dve", "act"]]

            for t in range(nt_out):
                cur = nit[t % 2]
                prv = nit[(t + 1) % 2]
                dyn = dynF[t % 2]
                if t < 3:
                    nc.sync.dma_start(cur[0:1, :], ivT[t:t + 1, :])
                    nc.sync.dma_start(outT[t:t + 1, :], cur[0:1, :])
                else:
                    for c in range(nchunk):
                        cs = slice(c * ch, (c + 1) * ch)
                        nc.vector.scalar_tensor_tensor(
                            cur[0:1, cs], racc[0:1, cs], c3t[0:1, :],
                            iv2[0:1, cs], op0=AluOp.mult, op1=AluOp.add)
                    nc.sync.dma_start(outT[t:t + 1, :], cur[0:1, :])
                if t > nsteps - 1:
                    continue

                # recurrent features (ni*S_NI | ni^2) -> dynF pair blocks
                for c in range(nchunk):
                    cs = slice(c * ch, (c + 1) * ch)
                    nc.vector.tensor_scalar_mul(dyn[0:1, 2 * c, :],
                                                 cur[0:1, cs], S_NI)
                    nc.scalar.activation(dyn[0:1, 2 * c + 1, :], cur[0:1, cs],
                                         ACTF.Square)

                ft = fpool.tile([NKI1, 2, bc], F8, tag="F")
                nc.sync.dma_start(ft[:], fdram[t])

                h1ps = []
                for c in range(nchunk):
                    row = []
                    for m in range(2):
                        h1p = hpp.tile([100, ch], FP, tag="hp")
                        nc.tensor.matmul(
                            h1p[:, :], w1p[:, :, m * MPAD:m * MPAD + 100],
                            ft[:, :, c * ch:(c + 1) * ch],
                            start=True, stop=False, perf_mode=DR)
                        nc.tensor.matmul(
                            h1p[:, :], w1dyn[:, :, m * MPAD:m * MPAD + 100],
                            dyn[0:1, 2 * c:2 * c + 2, :],
                            start=False, stop=True, perf_mode=DR)
                        row.append(h1p)
                    h1ps.append(row)

                for c in range(nchunk):
                    for m in range(2):
                        relu_op(RELU1_ENG[c][m],
                                h1t[c][t % 2][0:100, m, :],
                                h1ps[c][m][:, :], s1t[:, m:m + 1], 100)

                h2ps = []
                for c in range(nchunk):
                    row = []
                    for m in range(2):
                        h2p = hpp.tile([100, ch], FP, tag="hp")
                        nc.tensor.matmul(h2p[:, :],
                                         w2hi[:, :, m * MPAD:m * MPAD + 100],
                                         h1t[c][t % 2][:, :, :],
                                         start=True, stop=False, perf_mode=DR)
                        nc.tensor.matmul(h2p[:, :],
                                         w2lo[:, :, m * MPAD:m * MPAD + 100],
                                         h1t[c][t % 2][:, :, :],
                                         start=False, stop=True, perf_mode=DR)
                        row.append(h2p)
                    h2ps.append(row)

                for c in range(nchunk):
                    for m in range(2):
                        relu_op(RELU2_ENG[c][m],
                                h2t[c][t % 3][0:100, m, :],
                                h2ps[c][m][:, :], s2t[:, m:m + 1], 100)

                if t >= 2:
                    first_step = (t == 2)
                    last_step = (t == nsteps - 1)
                    terms = [(2, 0), (1, 1), (0, 2)]
                    for ti, (j, d) in enumerate(terms):
                        for c in range(nchunk):
                            cs = slice(c * ch, (c + 1) * ch)
                            h2v = h2t[c][(t - d) % 3]
                            st_ = first_step and ti == 0
                            sp_ = last_step and ti == len(terms) - 1
                            nc.tensor.matmul(racc[0:1, cs],
                                             akhi[:, :, j:j + 1], h2v[:, :, :],
                                             start=st_, stop=False,
                                             perf_mode=DR,
                                             skip_group_check=True)
                            nc.tensor.matmul(racc[0:1, cs],
                                             aklo[:, :, j:j + 1], h2v[:, :, :],
                                             start=False, stop=sp_,
                                             perf_mode=DR,
                                             skip_group_check=True)
    nc.compile()
    return nc


# ================= host-side preparation =================

def _fp8(x):
    import ml_dtypes
    return np.asarray(x, np.float32).astype(ml_dtypes.float8_e4m3fn)


def _fp8r(x):
    return _fp8(x).astype(np.float32)


def _hilo8(x):
    hi = _fp8r(x)
    lo = _fp8r(np.asarray(x, np.float32) - hi)
    return hi, lo


def _pow2_fit(maxval, target=104.0):
    if maxval <= 0:
        return 1.0
    return float(2.0 ** np.floor(np.log2(target / maxval)))


def prep_host_inputs(nr, iv, k, W1, b1, W2, b2, W3, b3, Ws, bs):
    W1 = np.asarray(W1, np.float32)
    b1 = np.asarray(b1, np.float32)
    W2 = np.asarray(W2, np.float32)
    b2 = np.asarray(b2, np.float32)
    W3c = np.asarray(W3, np.float32)[:, 0]
    a = (H * np.asarray(Ws, np.float64)).astype(np.float32)
    K0 = np.float32(H * (np.float64(b3[0]) * np.asarray(Ws, np.float64).sum()
                         + np.float64(bs[0])))

    perm = [0, 2, 3, 4, 6, 7]
    scl = np.array([1, 1 / (2 * H), 1, 1, 2, 1 / (2 * H)], np.float32)
    W1s = (W1[perm, :] * scl[:, None]).astype(np.float32)
    w1dn = W1[1, :]
    w1dq = W1[5, :]

    nr_ = np.asarray(nr, np.float32)
    nrmax = float(np.abs(nr_).max())
    dnr_max = 2.0 * nrmax + 1.0
    fmax = np.array([nrmax, dnr_max, 1.0, nrmax ** 2 + 1,
                     nrmax, nrmax * dnr_max], np.float32)
    NIMAX = 24.0
    h1max = (np.abs(W1s) * fmax[:, None]).sum(0) + np.abs(b1) \
        + np.abs(w1dn) * NIMAX + np.abs(w1dq) * NIMAX ** 2 + 1e-3
    s1 = np.array([_pow2_fit(v) for v in h1max], np.float32)

    h1b = h1max * s1                      # <= 224; h1t values bounded by this
    h2max = (np.abs(W2 / s1[:, None]) * h1b[:, None]).sum(0) + np.abs(b2) + 1e-3
    s2 = np.array([_pow2_fit(v) for v in h2max], np.float32)

    # ---- W1 static slot pairs (rows 1/5 carry /16; features carry x16) ----
    w1hi1, w1lo1 = _hilo8(W1s[1] / 16.0)
    w1hi5, w1lo5 = _hilo8(W1s[5] / 8.0)
    slots = [
        (w1hi1, w1hi1 / 16.0),
        (w1lo1, w1hi5),
        (w1hi5 / 16.0, w1lo5),
        (W1s[0], W1s[2]),
        (W1s[3], W1s[4]),
        (b1, np.zeros(200, np.float32)),
    ]
    W1P = np.zeros((NKI1, 2, 2 * MPAD), np.float32)
    for ki, (wa, wb) in enumerate(slots):
        for m in range(2):
            W1P[ki, 0, m * MPAD:m * MPAD + 100] = wa[m * 100:(m + 1) * 100]
            W1P[ki, 1, m * MPAD:m * MPAD + 100] = wb[m * 100:(m + 1) * 100]
    W1P = _fp8(W1P)

    W1DYN = np.zeros((1, 2, 2 * MPAD), np.float32)
    for m in range(2):
        W1DYN[0, 0, m * MPAD:m * MPAD + 100] = w1dn[m * 100:(m + 1) * 100] / S_NI
        W1DYN[0, 1, m * MPAD:m * MPAD + 100] = w1dq[m * 100:(m + 1) * 100]
    W1DYN = _fp8(W1DYN)

    # ---- W2 pairs ----
    W2c = W2 / s1[:, None]
    c2 = _pow2_fit(float(np.abs(W2c).max()), 104.0)
    W2cc = W2c * c2
    W2HI = np.zeros((101, 2, 2 * MPAD), np.float32)
    W2LO = np.zeros((101, 2, 2 * MPAD), np.float32)
    hi2, lo2 = _hilo8(W2cc)               # [200, 200]
    for i in range(2):
        for m in range(2):
            W2HI[0:100, i, m * MPAD:m * MPAD + 100] = \
                hi2[100 * i:100 * (i + 1), m * 100:(m + 1) * 100]
            W2LO[0:100, i, m * MPAD:m * MPAD + 100] = \
                lo2[100 * i:100 * (i + 1), m * 100:(m + 1) * 100]
    for m in range(2):
        W2HI[100, 0, m * MPAD:m * MPAD + 100] = b2[m * 100:(m + 1) * 100] * c2
    W2HI = _fp8(W2HI)
    W2LO = _fp8(W2LO)

    # ---- AK pairs ----
    AKHI = np.zeros((101, 2, 16), np.float32)
    AKLO = np.zeros((101, 2, 16), np.float32)
    s2c2 = s2 / c2
    akmax = float(np.abs(a[:, None] * W3c[None, :] / s2[None, :]).max())
    akmax = max(akmax, abs(float(K0)) + 1e-30)
    c3 = 2.0 ** np.ceil(np.log2(akmax / 104.0))
    for j in range(3):
        col = a[j] * W3c / s2 / c3
        hi, lo = _hilo8(col)
        for i in range(2):
            AKHI[0:100, i, j] = hi[100 * i:100 * (i + 1)]
            AKLO[0:100, i, j] = lo[100 * i:100 * (i + 1)]
    AKHI[100, 0, 2] = K0 / c3
    AKHI = _fp8(AKHI)
    AKLO = _fp8(AKLO)

    S1 = np.stack([s1[0:100], s1[100:200]], 1).astype(np.float32)
    S2 = np.stack([s2c2[0:100], s2c2[100:200]], 1).astype(np.float32)
    ONE8D = np.zeros((1, 2, CH), np.float32)
    ONE8D[0, 0, :] = 1.0
    return {
        "W1P": W1P, "W1DYN": W1DYN, "W2HI": W2HI, "W2LO": W2LO,
        "AKHI": AKHI, "AKLO": AKLO, "S1": S1, "S2": S2, "ONE8D": _fp8(ONE8D),
        "C3V": np.full((1, 1), c3, np.float32),
    }


_CACHED_NC = None


def _get_nc():
    global _CACHED_NC
    if _CACHED_NC is None:
        _CACHED_NC = build_program()
    return _CACHED_NC


def make_in_maps(inputs):
    nr = np.asarray(inputs["nr"], np.float32)
    iv = np.asarray(inputs["iv"], np.float32)
    k = np.asarray(inputs["k"], np.float32)
    shared = prep_host_inputs(nr, iv, k, inputs["W1"], inputs["b1"],
                              inputs["W2"], inputs["b2"], inputs["W3"],
                              inputs["b3"], inputs["Ws"], inputs["bs"])
    in_maps = []
    for i in range(NCORES):
        sl = slice(i * BC, (i + 1) * BC)
        m = dict(shared)
        m["nrT"] = np.ascontiguousarray(nr[sl].T)
        m["ivT"] = np.ascontiguousarray(iv[sl].T)
        m["kb"] = np.ascontiguousarray(
            np.broadcast_to(k[sl][None, :], (128, BC)))
        in_maps.append(m)
    return in_maps


def run(inputs, trace=False):
    from concourse.bass_utils import run_bass_kernel_spmd

    assert int(inputs["stop"]) == 512
    in_maps = make_in_maps(inputs)
    nc = _get_nc()
    res = run_bass_kernel_spmd(nc, in_maps, list(range(NCORES)), trace=trace)
    out = np.concatenate([res.results[i]["outT"].T for i in range(NCORES)], 0)
    return out.astype(np.float32), res


def kernel(**inputs):
    out, _ = run(inputs, trace=False)
    return out


def time_hw(inputs, iters=5):
    """Device-resident timing (no host transfers in the timed region)."""
    import time
    import jax
    from jax.sharding import Mesh, PartitionSpec, NamedSharding
    from jax.experimental.shard_map import shard_map
    from concourse import bass2jax

    in_maps = make_in_maps(inputs)
    nc = _get_nc()
    bass2jax.install_neuronx_cc_hook()
    import concourse.mybir as mb
    pname = nc.partition_id_tensor.name if nc.partition_id_tensor else None
    in_names, out_names, out_avals, zero_outs = [], [], [], []
    for alloc in nc.m.functions[0].allocations:
        if not isinstance(mb.MemoryLocationSet, type) or not isinstance(
                alloc, mb.MemoryLocationSet):
            continue
        name = alloc.memorylocations[0].name
        if alloc.kind == "ExternalInput":
            if name != pname:
                in_names.append(name)
        elif alloc.kind == "ExternalOutput":
            out_names.append(name)
            shape = tuple(alloc.tensor_shape)
            dtype = mb.dt.np(alloc.dtype)
            out_avals.append(jax.core.ShapedArray(shape, dtype))
            zero_outs.append(np.zeros(shape, dtype))
    n_params = len(in_names)
    all_in = list(in_names) + list(out_names)
    if pname is not None:
        all_in.append(pname)

    def _body(*args):
        operands = list(args)
        if pname is not None:
            operands.append(bass2jax.partition_id_tensor())
        outs = bass2jax._bass_exec_p.bind(
            *operands,
            out_avals=tuple(out_avals),
            in_names=tuple(all_in),
            out_names=tuple(out_names),
            lowering_input_output_aliases=(),
            sim_require_finite=True,
            sim_require_nnan=True,
            nc=nc,
        )
        return tuple(outs)

    devices = jax.devices()[:NCORES]
    mesh = Mesh(np.asarray(devices), ("core",))
    specs = (PartitionSpec("core"),)
    fn = jax.jit(shard_map(_body, mesh=mesh,
                           in_specs=specs * (n_params + len(out_names)),
                           out_specs=specs * len(out_names),
                           check_rep=False))
    concat_in = [np.concatenate([np.asarray(in_maps[c][nm])
                                 for c in range(NCORES)], 0)
                 for nm in in_names]
    concat_zeros = [np.zeros((NCORES * z.shape[0], *z.shape[1:]), z.dtype)
                    for z in zero_outs]
    sh = NamedSharding(mesh, PartitionSpec("core"))
    dev_in = [jax.device_put(a, sh) for a in concat_in + concat_zeros]
    out = fn(*dev_in)
    jax.block_until_ready(out)
    times = []
    for _ in range(iters):
        t0 = time.perf_counter()
        out = fn(*dev_in)
        jax.block_until_ready(out)
        times.append(time.perf_counter() - t0)
    full = np.asarray(out[0]).reshape(NCORES, NT_OUT, BC)
    res = np.concatenate([full[c].T for c in range(NCORES)], 0)
    return res.astype(np.float32), times
